# revision 29
# baseline (speedup 1.0000x reference)
"""DFlash Qwen3 cross-attention on 8 TRN2 NeuronCores.

Sharding: tensor-parallel over heads. Core c owns KV head c (KVH=8) and the
4 query heads 4c..4c+3 of its GQA group. Each core computes its heads'
QKV projections, per-head RMSNorm + RoPE, causal attention, then the
normalized per-head attention outputs ([4*D, QL] transposed) are
AllGathered so every core holds attn^T [H*D, QL]; each core then computes a
512-column slice of o_proj and the host concatenates the 8 slices.

Matmuls run in fp16 (fp32 PSUM accumulation); softmax in fp32 on the scalar
engine with 2-bank-batched EXP reads; softmax denominators accumulate in
fp16 on the vector engine (2x packed mode) and are broadcast across
partitions by a ones-matmul on the tensor engine.

Host-side prep: transpose ck=concat(context,query) to [HID, KV] fp16,
slice per-core weights, precompute RoPE cos/sin (with the per-head norm
weights folded in) and causal {0,1} mask tiles.
"""

from contextlib import ExitStack

import numpy as np

import concourse.bass as bass
import concourse.mybir as mybir
import concourse.tile as tile
from concourse import bacc
from concourse.bass_utils import run_bass_kernel_spmd

H = 32
KVH = 8
D = 128
HID = 4096
CTX = 4096
QL = 2048
KV = CTX + QL  # 6144
NCORES = 8
HPC = H // NCORES  # 4 q heads per core
THETA = 1000000.0
EPS = 1e-6
SCALE = float(D) ** -0.5

NHD = HID // 128  # 32 contraction chunks
NKV = KV // 128  # 48 kv chunks
NQC = QL // 128  # 16 q row chunks
NQJ = QL // 512  # 4 q column tiles for attention

F32 = mybir.dt.float32
F16 = mybir.dt.float16

_STATE = {}


def _build():
    nc = bacc.Bacc()

    # chunk-major ck^T: ckT[r, p, k, c] = ck[r*128+c, k*128+p] so one kv-chunk
    # load is a contiguous 8KB-per-partition DMA
    ckT = nc.declare_dram_parameter("ckT", [NKV, 128, NHD, 128], F16, isOutput=False)
    # weights pre-shuffled to [p, k, n] so the resident loads are contiguous
    wq = nc.declare_dram_parameter("wq", [128, NHD, HPC * D], F16, isOutput=False)
    wkv = nc.declare_dram_parameter("wkv", [128, NHD, 2 * D], F16, isOutput=False)
    wo = nc.declare_dram_parameter("wo", [128, NHD, HPC * D], F16, isOutput=False)
    # cs4: [KV, 4*64] = cos*w1 | sin*w2 | cos*w2 | sin*w1 per position
    # (q-norm weights folded for rows >= CTX ... actually separate q/k tables)
    csq = nc.declare_dram_parameter("csq", [QL, 4 * 64], F32, isOutput=False)
    csk = nc.declare_dram_parameter("csk", [KV, 4 * 64], F32, isOutput=False)
    msk = nc.declare_dram_parameter("msk", [128, 4 * 512], F16, isOutput=False)
    out_ext = nc.declare_dram_parameter("out", [QL, HPC * D], F32, isOutput=True)

    # per-(head, j) AllGather buffers: fire as each strip finishes.
    ag_ins = [[nc.dram_tensor(f"ag_in{h}_{j}", [D, 512], F16) for j in range(NQJ)]
              for h in range(HPC)]
    ag_outs = [[nc.dram_tensor(f"ag_out{h}_{j}", [NCORES * D, 512], F16,
                               addr_space="Shared") for j in range(NQJ)]
               for h in range(HPC)]

    with tile.TileContext(nc) as tc, ExitStack() as ctx:
        singles = ctx.enter_context(tc.tile_pool(name="singles", bufs=1))
        ck_pool = ctx.enter_context(tc.tile_pool(name="ckp", bufs=3))
        cs_pool = ctx.enter_context(tc.tile_pool(name="csp", bufs=3))
        evac_pool = ctx.enter_context(tc.tile_pool(name="evac", bufs=2))
        tmp_pool = ctx.enter_context(tc.tile_pool(name="tmp", bufs=2))
        pt_pool = ctx.enter_context(tc.tile_pool(name="pt", bufs=4))
        sacc_pool = ctx.enter_context(tc.tile_pool(name="sacc", bufs=2))
        stg_pool = ctx.enter_context(tc.tile_pool(name="stg", bufs=2))
        oproj_pool = ctx.enter_context(tc.tile_pool(name="oproj", bufs=3))
        # PSUM: tag "acc" 3x 1-bank accumulators, tag "tp" 1 transpose bank,
        # tag "st" 2x 2-bank S tiles = 8 banks
        psum = ctx.enter_context(tc.tile_pool(name="psum", bufs=3, space="PSUM"))

        # ---- resident tensors ----
        wq_sb = singles.tile([128, NHD, HPC * D], F16)
        nc.sync.dma_start(out=wq_sb[:], in_=wq[:, :, :])
        wkv_sb = singles.tile([128, NHD, 2 * D], F16)
        nc.sync.dma_start(out=wkv_sb[:], in_=wkv[:, :, :])
        wo_sb = singles.tile([128, NHD, HPC * D], F16)
        nc.gpsimd.dma_start(out=wo_sb[:], in_=wo[:, :, :])
        msk_sb = singles.tile([128, 4, 512], F16)
        nc.gpsimd.dma_start(out=msk_sb[:], in_=msk[:, :].rearrange("p (i c) -> p i c", c=512))

        epst = singles.tile([128, 1], F32)
        nc.vector.memset(epst, EPS)
        zbias = singles.tile([128, 1], F32)
        nc.vector.memset(zbias, 0.0)
        ones128 = singles.tile([128, 128], F16)
        nc.vector.memset(ones128, 1.0)
        from concourse.masks import make_identity
        ident16 = singles.tile([128, 128], F16)
        make_identity(nc, ident16)



        qT_sb = singles.tile([128, HPC, QL], F16)  # Q^T per head: [d, h, q]
        kT_sb = singles.tile([128, KV], F16)  # K^T: [d, kv]
        v_sb = singles.tile([128, NKV, D], F16)  # V: [kv%128, r, d]

        def rmsnorm_rope_k(ke, cst, ro16):
            """ke: [128,256] f32 (K|V for one kv chunk); norm+rope K -> ro16 f16."""
            kx = ke[:, 0:D]
            sq = tmp_pool.tile([128, D], F32, tag="sq")
            nc.vector.tensor_mul(sq, kx, kx)
            ssum = tmp_pool.tile([128, 1], F32, tag="ssum")
            nc.vector.tensor_reduce(ssum, sq, axis=mybir.AxisListType.X,
                                    op=mybir.AluOpType.add)
            nc.scalar.activation(out=ssum, in_=ssum,
                                 func=mybir.ActivationFunctionType.Sqrt,
                                 bias=epst, scale=1.0 / D)
            nc.vector.reciprocal(ssum, ssum)
            nc.vector.tensor_scalar_mul(out=kx, in0=kx, scalar1=ssum)
            x1 = kx[:, 0:64]
            x2 = kx[:, 64:128]
            cw1 = cst[:, 0:64]
            sw2 = cst[:, 64:128]
            cw2 = cst[:, 128:192]
            sw1 = cst[:, 192:256]
            t1 = tmp_pool.tile([128, 64], F32, tag="t1")
            t2 = tmp_pool.tile([128, 64], F32, tag="t2")
            nc.vector.tensor_mul(t1, x1, cw1)
            nc.vector.tensor_mul(t2, x2, sw2)
            nc.vector.tensor_sub(ro16[:, 0:64], t1, t2)
            nc.vector.tensor_mul(t1, x2, cw2)
            nc.vector.tensor_mul(t2, x1, sw1)
            nc.vector.tensor_add(ro16[:, 64:128], t1, t2)

        def rmsnorm_rope_q4(qe, cst, ro16):
            """qe: [128, 4*128] f32 (4 heads); norm+rope all heads -> ro16 [128,4,128] f16."""
            q4 = qe.rearrange("p (h d) -> p h d", d=D)
            sq = tmp_pool.tile([128, 4, D], F32, tag="sq4")
            nc.vector.tensor_mul(sq, q4, q4)
            ssum = tmp_pool.tile([128, 4], F32, tag="ssum4")
            nc.vector.tensor_reduce(ssum, sq, axis=mybir.AxisListType.X,
                                    op=mybir.AluOpType.add)
            nc.scalar.activation(out=ssum, in_=ssum,
                                 func=mybir.ActivationFunctionType.Sqrt,
                                 bias=epst, scale=1.0 / D)
            nc.vector.reciprocal(ssum, ssum)
            sb = tmp_pool.tile([128, 4, D], F32, tag="sb4")
            nc.vector.tensor_copy(out=sb, in_=ssum[:, :, None].broadcast_to([128, 4, D]))
            nc.vector.tensor_mul(qe, qe, sb.rearrange("p h d -> p (h d)"))
            x1 = q4[:, :, 0:64]
            x2 = q4[:, :, 64:128]
            cw1 = cst[:, None, 0:64].broadcast_to([128, 4, 64])
            sw2 = cst[:, None, 64:128].broadcast_to([128, 4, 64])
            cw2 = cst[:, None, 128:192].broadcast_to([128, 4, 64])
            sw1 = cst[:, None, 192:256].broadcast_to([128, 4, 64])
            t1 = tmp_pool.tile([128, 4, 64], F32, tag="t14")
            t2 = tmp_pool.tile([128, 4, 64], F32, tag="t24")
            nc.vector.tensor_mul(t1, x1, cw1)
            nc.vector.tensor_mul(t2, x2, sw2)
            nc.vector.tensor_sub(ro16[:, :, 0:64], t1, t2)
            nc.vector.tensor_mul(t1, x2, cw2)
            nc.vector.tensor_mul(t2, x1, sw1)
            nc.vector.tensor_add(ro16[:, :, 64:128], t1, t2)

        # ---- phase 1: query columns -> Q proj (all 4 heads) + KV rows 32+qc ----
        for qc in range(NQC):
            ckt = ck_pool.tile([128, NHD, 128], F16, tag="ck")
            nc.sync.dma_start(out=ckt, in_=ckT[NKV - NQC + qc])
            pq = psum.tile([128, HPC * D], F32, tag="acc", name=f"pq{qc}")
            pk = psum.tile([128, 2 * D], F32, tag="acc", name=f"pk{qc}")
            for k in range(NHD):
                nc.tensor.matmul(pq, lhsT=ckt[:, k, :], rhs=wq_sb[:, k, :],
                                 start=(k == 0), stop=(k == NHD - 1))
            for k in range(NHD):
                nc.tensor.matmul(pk, lhsT=ckt[:, k, :], rhs=wkv_sb[:, k, :],
                                 start=(k == 0), stop=(k == NHD - 1))
            r = NKV - NQC + qc  # kv chunk index of this query block

            cstq = cs_pool.tile([128, 4 * 64], F32, tag="csq")
            nc.scalar.dma_start(out=cstq, in_=csq[qc * 128:(qc + 1) * 128, :])
            cstk = cs_pool.tile([128, 4 * 64], F32, tag="csk")
            nc.scalar.dma_start(out=cstk, in_=csk[r * 128:(r + 1) * 128, :])

            qe = evac_pool.tile([128, HPC * D], F32, tag="qe")
            nc.scalar.copy(out=qe, in_=pq)
            ke = evac_pool.tile([128, 2 * D], F32, tag="ke")
            nc.scalar.copy(out=ke, in_=pk)

            ro4 = tmp_pool.tile([128, 4, 128], F16, tag="ro4")
            rmsnorm_rope_q4(qe, cstq, ro4)
            tpq = psum.tile([128, 512], F16, tag="tp", bufs=1, name=f"tpq{qc}")
            for h in range(HPC):
                nc.tensor.transpose(tpq[:, h * 128:(h + 1) * 128], ro4[:, h, :], ident16)
                nc.vector.tensor_copy(out=qT_sb[:, h, qc * 128:(qc + 1) * 128],
                                      in_=tpq[:, h * 128:(h + 1) * 128])
            rok = tmp_pool.tile([128, 128], F16, tag="rok")
            rmsnorm_rope_k(ke, cstk, rok)
            tpk = psum.tile([128, 512], F16, tag="tp", bufs=1, name=f"tpk{qc}")
            nc.tensor.transpose(tpk[:, 0:128], rok, ident16)
            nc.vector.tensor_copy(out=kT_sb[:, r * 128:(r + 1) * 128], in_=tpk[:, 0:128])
            nc.vector.tensor_copy(out=v_sb[:, r, :], in_=ke[:, D:2 * D])

        # ---- phase 2: context rows -> K/V chunks r=0..31 ----
        for r in range(NKV - NQC):
            ckt = ck_pool.tile([128, NHD, 128], F16, tag="ck")
            nc.sync.dma_start(out=ckt, in_=ckT[r])
            pk = psum.tile([128, 2 * D], F32, tag="acc", name=f"pkc{r}")
            for k in range(NHD):
                nc.tensor.matmul(pk, lhsT=ckt[:, k, :], rhs=wkv_sb[:, k, :],
                                 start=(k == 0), stop=(k == NHD - 1))
            cstk = cs_pool.tile([128, 4 * 64], F32, tag="csk")
            nc.scalar.dma_start(out=cstk, in_=csk[r * 128:(r + 1) * 128, :])
            ke = evac_pool.tile([128, 2 * D], F32, tag="ke")
            nc.scalar.copy(out=ke, in_=pk)
            rok = tmp_pool.tile([128, 128], F16, tag="rok")
            rmsnorm_rope_k(ke, cstk, rok)
            tpk = psum.tile([128, 512], F16, tag="tp", bufs=1, name=f"tpc{r}")
            nc.tensor.transpose(tpk[:, 0:128], rok, ident16)
            nc.vector.tensor_copy(out=kT_sb[:, r * 128:(r + 1) * 128], in_=tpk[:, 0:128])
            nc.vector.tensor_copy(out=v_sb[:, r, :], in_=ke[:, D:2 * D])

        # ---- phase 3: attention strips (j outer, h inner) + interleaved o_proj ----
        # S^T orientation: [kv partitions, q free]; exp output IS P^T; PV with
        # V stationary gives out^T [d, q].  kv chunk r fully visible iff
        # r <= 31+4j, partial (mask i=r-32-4j) for i in 0..3.
        def do_oproj(qc):
            po = psum.tile([128, HPC * D], F32, tag="acc", name=f"po{qc}")
            jq, qo = qc // 4, (qc % 4) * 128
            for h in range(HPC):
                at = oproj_pool.tile([128, NCORES, 128], F16, tag="at")
                nc.sync.dma_start(
                    out=at,
                    in_=ag_outs[h][jq][:, qo:qo + 128].rearrange(
                        "(c p) q -> p c q", p=128))
                for ci in range(NCORES):
                    nc.tensor.matmul(po, lhsT=at[:, ci, :], rhs=wo_sb[:, 4 * ci + h, :],
                                     start=(h == 0 and ci == 0),
                                     stop=(h == HPC - 1 and ci == NCORES - 1))
            ot = stg_pool.tile([128, HPC * D], F32, tag="ot")
            nc.vector.tensor_copy(out=ot, in_=po)
            nc.sync.dma_start(out=out_ext[qc * 128:(qc + 1) * 128, :], in_=ot)

        def do_strip(h, j):
            n_r = 36 + 4 * j
            o_acc = psum.tile([128, 512], F32, tag="acc", name=f"oacc{h}_{j}")
            sacc = sacc_pool.tile([128, 2, 512], F16, tag="sacc", name=f"sacc{h}_{j}")
            qs = qT_sb[:, h, j * 512:(j + 1) * 512]
            for b in range(n_r // 2):
                r0 = 2 * b
                st2 = psum.tile([128, 2, 512], F32, tag="st", bufs=2,
                                name=f"st{h}_{j}_{b}")
                for i in range(2):
                    nc.tensor.matmul(st2[:, i, :], lhsT=kT_sb[:, (r0 + i) * 128:(r0 + i + 1) * 128],
                                     rhs=qs, start=True, stop=True)
                pt2 = pt_pool.tile([128, 2, 512], F16, tag="pt")
                nc.scalar.activation(out=pt2.rearrange("p i c -> p (i c)"),
                                     in_=st2.rearrange("p i c -> p (i c)"),
                                     func=mybir.ActivationFunctionType.Exp,
                                     bias=zbias, scale=SCALE)
                i0 = r0 - 32 - 4 * j  # mask index of first tile in batch
                if i0 >= 0:
                    nc.vector.tensor_mul(pt2.rearrange("p i c -> p (i c)"),
                                         pt2.rearrange("p i c -> p (i c)"),
                                         msk_sb[:, i0:i0 + 2, :].rearrange("p i c -> p (i c)"))
                if b == 0:
                    nc.vector.tensor_copy(out=sacc, in_=pt2)
                else:
                    nc.vector.tensor_add(sacc.rearrange("p i c -> p (i c)"),
                                         sacc.rearrange("p i c -> p (i c)"),
                                         pt2.rearrange("p i c -> p (i c)"))
                for i in range(2):
                    nc.tensor.matmul(o_acc, lhsT=v_sb[:, r0 + i, :], rhs=pt2[:, i, :],
                                     start=(r0 + i == 0), stop=(r0 + i == n_r - 1))
            # denominator: broadcast column sums via ones-matmul, then normalize
            denom = psum.tile([128, 512], F32, tag="tp", bufs=1, name=f"den{h}_{j}")
            for i in range(2):
                nc.tensor.matmul(denom, lhsT=ones128, rhs=sacc[:, i, :],
                                 start=(i == 0), stop=(i == 1))
            pr = stg_pool.tile([128, 512], F32, tag="pr")
            nc.vector.reciprocal(pr, denom)
            stg = stg_pool.tile([128, 512], F16, tag="stg")
            nc.vector.tensor_mul(stg, o_acc, pr)
            nc.sync.dma_start(out=ag_ins[h][j][:], in_=stg)
            nc.gpsimd.collective_compute(
                "AllGather",
                mybir.AluOpType.bypass,
                ins=[ag_ins[h][j][:]],
                outs=[ag_outs[h][j][:]],
                replica_groups=[list(range(NCORES))],
            )

        # o_proj chunk 4*(j-1)+h is emitted right after strip (h, j): the
        # previous j-group's o_proj spreads across this group's strips.
        for j in range(NQJ):
            for h in range(HPC):
                do_strip(h, j)
                if j >= 1:
                    do_oproj(4 * (j - 1) + h)
        for qc in range(4 * (NQJ - 1), NQC):
            do_oproj(qc)

    nc.compile()
    return nc


def _host_prep(context, query, w_qkv, w_o, q_norm_w, k_norm_w):
    context = np.asarray(context, dtype=np.float32)
    query = np.asarray(query, dtype=np.float32)
    w_qkv = np.asarray(w_qkv, dtype=np.float32)
    w_o = np.asarray(w_o, dtype=np.float32)
    q_norm_w = np.asarray(q_norm_w, dtype=np.float32)
    k_norm_w = np.asarray(k_norm_w, dtype=np.float32)

    ck = np.concatenate([context, query], axis=0)  # [KV, HID]
    # chunk-major ckT[r, p, k, c] = ck[r*128+c, k*128+p]
    ckT = np.ascontiguousarray(
        ck.astype(np.float16).reshape(NKV, 128, NHD, 128).transpose(0, 3, 2, 1))

    wq = w_qkv[:, :H * D]
    wk = w_qkv[:, H * D:H * D + KVH * D]
    wv = w_qkv[:, H * D + KVH * D:]

    half = D // 2
    inv_freq = (1.0 / (THETA ** (np.arange(0, half, dtype=np.float32) / half))).astype(np.float32)
    pos = np.arange(KV, dtype=np.float32)
    freqs = pos[:, None] * inv_freq[None, :]
    cosf = np.cos(freqs)
    sinf = np.sin(freqs)

    def cs4(w):
        w1 = w[:half][None, :]
        w2 = w[half:][None, :]
        return np.concatenate([cosf * w1, sinf * w2, cosf * w2, sinf * w1],
                              axis=1).astype(np.float32)  # [KV, 256]

    csq = np.ascontiguousarray(cs4(q_norm_w)[CTX:])  # [QL, 256]
    csk = cs4(k_norm_w)  # [KV, 256]

    p = np.arange(128)[:, None]
    q = np.arange(512)[None, :]
    msk = np.concatenate(
        [np.where(128 * i + p <= q, 1.0, 0.0) for i in range(4)],
        axis=1).astype(np.float16)  # [128, 2048]

    def wshuf(w):
        # [HID, n] -> [128, NHD, n] with w[k*128+p, :] at [p, k, :]
        n = w.shape[1]
        return np.ascontiguousarray(
            w.astype(np.float16).reshape(NHD, 128, n).transpose(1, 0, 2))

    in_maps = []
    for c in range(NCORES):
        in_maps.append({
            "ckT": ckT,
            "wq": wshuf(wq[:, c * HPC * D:(c + 1) * HPC * D]),
            "wkv": wshuf(np.concatenate(
                [wk[:, c * D:(c + 1) * D], wv[:, c * D:(c + 1) * D]], axis=1)),
            "wo": wshuf(w_o[:, c * HPC * D:(c + 1) * HPC * D]),
            "csq": csq,
            "csk": csk,
            "msk": msk,
        })
    return in_maps


def kernel(context, query, w_qkv, w_o, q_norm_w, k_norm_w, **kw):
    if "nc" not in _STATE:
        _STATE["nc"] = _build()
    nc = _STATE["nc"]
    in_maps = _host_prep(context, query, w_qkv, w_o, q_norm_w, k_norm_w)
    res = run_bass_kernel_spmd(nc, in_maps, list(range(NCORES)), **kw)
    out = np.concatenate([np.asarray(res.results[c]["out"]) for c in range(NCORES)], axis=1)
    if kw:
        return out.astype(np.float32), res
    return out.astype(np.float32)


# revision 30
# speedup vs baseline: 1.0756x; 1.0756x over previous
"""DFlash Qwen3 cross-attention on 8 TRN2 NeuronCores.

Sharding: tensor-parallel over heads. Core c owns KV head c (KVH=8) and the
4 query heads 4c..4c+3 of its GQA group. Each core computes its heads'
QKV projections, per-head RMSNorm + RoPE, causal attention, then the
normalized per-head attention outputs ([4*D, QL] transposed) are
AllGathered so every core holds attn^T [H*D, QL]; each core then computes a
512-column slice of o_proj and the host concatenates the 8 slices.

Matmuls run in fp16 (fp32 PSUM accumulation); softmax in fp32 on the scalar
engine with 2-bank-batched EXP reads; softmax denominators accumulate in
fp16 on the vector engine (2x packed mode) and are broadcast across
partitions by a ones-matmul on the tensor engine.

Host-side prep: transpose ck=concat(context,query) to [HID, KV] fp16,
slice per-core weights, precompute RoPE cos/sin (with the per-head norm
weights folded in) and causal {0,1} mask tiles.
"""

from contextlib import ExitStack

import numpy as np

import concourse.bass as bass
import concourse.mybir as mybir
import concourse.tile as tile
from concourse import bacc
from concourse.bass_utils import run_bass_kernel_spmd

H = 32
KVH = 8
D = 128
HID = 4096
CTX = 4096
QL = 2048
KV = CTX + QL  # 6144
NCORES = 8
HPC = H // NCORES  # 4 q heads per core
THETA = 1000000.0
EPS = 1e-6
SCALE = float(D) ** -0.5

NHD = HID // 128  # 32 contraction chunks
NKV = KV // 128  # 48 kv chunks
NQC = QL // 128  # 16 q row chunks
NQJ = QL // 512  # 4 q column tiles for attention

F32 = mybir.dt.float32
F16 = mybir.dt.float16

_STATE = {}


def _build():
    nc = bacc.Bacc()

    # chunk-major ck^T: ckT[r, p, k, c] = ck[r*128+c, k*128+p] so one kv-chunk
    # load is a contiguous 8KB-per-partition DMA
    ckT = nc.declare_dram_parameter("ckT", [NKV, 128, NHD, 128], F16, isOutput=False)
    # weights pre-shuffled to [p, k, n] so the resident loads are contiguous
    wq = nc.declare_dram_parameter("wq", [128, NHD, HPC * D], F16, isOutput=False)
    wkv = nc.declare_dram_parameter("wkv", [128, NHD, 2 * D], F16, isOutput=False)
    wo = nc.declare_dram_parameter("wo", [128, NHD, HPC * D], F16, isOutput=False)
    # cs4: [KV, 4*64] = cos*w1 | sin*w2 | cos*w2 | sin*w1 per position
    # (q-norm weights folded for rows >= CTX ... actually separate q/k tables)
    csq = nc.declare_dram_parameter("csq", [QL, 4 * 64], F32, isOutput=False)
    csk = nc.declare_dram_parameter("csk", [KV, 4 * 64], F32, isOutput=False)
    msk = nc.declare_dram_parameter("msk", [128, 4 * 512], F16, isOutput=False)
    out_ext = nc.declare_dram_parameter("out", [QL, HPC * D], F32, isOutput=True)

    # per-(head, j) AllGather buffers: fire as each strip finishes.
    ag_ins = [[nc.dram_tensor(f"ag_in{h}_{j}", [D, 512], F16) for j in range(NQJ)]
              for h in range(HPC)]
    ag_outs = [[nc.dram_tensor(f"ag_out{h}_{j}", [NCORES * D, 512], F16,
                               addr_space="Shared") for j in range(NQJ)]
               for h in range(HPC)]

    with tile.TileContext(nc) as tc, ExitStack() as ctx:
        singles = ctx.enter_context(tc.tile_pool(name="singles", bufs=1))
        ck_pool = ctx.enter_context(tc.tile_pool(name="ckp", bufs=3))
        cs_pool = ctx.enter_context(tc.tile_pool(name="csp", bufs=3))
        evac_pool = ctx.enter_context(tc.tile_pool(name="evac", bufs=2))
        tmp_pool = ctx.enter_context(tc.tile_pool(name="tmp", bufs=2))
        pt_pool = ctx.enter_context(tc.tile_pool(name="pt", bufs=4))
        sacc_pool = ctx.enter_context(tc.tile_pool(name="sacc", bufs=2))
        stg_pool = ctx.enter_context(tc.tile_pool(name="stg", bufs=2))
        oproj_pool = ctx.enter_context(tc.tile_pool(name="oproj", bufs=3))
        # PSUM: tag "acc" 3x 1-bank accumulators, tag "tp" 1 transpose bank,
        # tag "st" 2x 2-bank S tiles = 8 banks
        psum = ctx.enter_context(tc.tile_pool(name="psum", bufs=3, space="PSUM"))

        # ---- resident tensors ----
        wq_sb = singles.tile([128, NHD, HPC * D], F16)
        nc.sync.dma_start(out=wq_sb[:], in_=wq[:, :, :])
        wkv_sb = singles.tile([128, NHD, 2 * D], F16)
        nc.sync.dma_start(out=wkv_sb[:], in_=wkv[:, :, :])
        wo_sb = singles.tile([128, NHD, HPC * D], F16)
        nc.gpsimd.dma_start(out=wo_sb[:], in_=wo[:, :, :])
        msk_sb = singles.tile([128, 4, 512], F16)
        nc.gpsimd.dma_start(out=msk_sb[:], in_=msk[:, :].rearrange("p (i c) -> p i c", c=512))

        epst = singles.tile([128, 1], F32)
        nc.vector.memset(epst, EPS)
        zbias = singles.tile([128, 1], F32)
        nc.vector.memset(zbias, 0.0)
        ones128 = singles.tile([128, 128], F16)
        nc.vector.memset(ones128, 1.0)
        from concourse.masks import make_identity
        ident16 = singles.tile([128, 128], F16)
        make_identity(nc, ident16)



        qT_sb = singles.tile([128, HPC, QL], F16)  # Q^T per head: [d, h, q]
        kT_sb = singles.tile([128, KV], F16)  # K^T: [d, kv]
        v_sb = singles.tile([128, NKV, D], F16)  # V: [kv%128, r, d]

        def rmsnorm_rope_k(ke, cst, ro16):
            """ke: [128,256] f32 (K|V for one kv chunk); norm+rope K -> ro16 f16."""
            kx = ke[:, 0:D]
            sq = tmp_pool.tile([128, D], F32, tag="sq")
            nc.vector.tensor_mul(sq, kx, kx)
            ssum = tmp_pool.tile([128, 1], F32, tag="ssum")
            nc.vector.tensor_reduce(ssum, sq, axis=mybir.AxisListType.X,
                                    op=mybir.AluOpType.add)
            nc.scalar.activation(out=ssum, in_=ssum,
                                 func=mybir.ActivationFunctionType.Sqrt,
                                 bias=epst, scale=1.0 / D)
            nc.vector.reciprocal(ssum, ssum)
            nc.vector.tensor_scalar_mul(out=kx, in0=kx, scalar1=ssum)
            x1 = kx[:, 0:64]
            x2 = kx[:, 64:128]
            cw1 = cst[:, 0:64]
            sw2 = cst[:, 64:128]
            cw2 = cst[:, 128:192]
            sw1 = cst[:, 192:256]
            t1 = tmp_pool.tile([128, 64], F32, tag="t1")
            t2 = tmp_pool.tile([128, 64], F32, tag="t2")
            nc.vector.tensor_mul(t1, x1, cw1)
            nc.vector.tensor_mul(t2, x2, sw2)
            nc.vector.tensor_sub(ro16[:, 0:64], t1, t2)
            nc.vector.tensor_mul(t1, x2, cw2)
            nc.vector.tensor_mul(t2, x1, sw1)
            nc.vector.tensor_add(ro16[:, 64:128], t1, t2)

        def rmsnorm_rope_q4(qe, cst, ro16):
            """qe: [128, 4*128] f32 (4 heads); norm+rope all heads -> ro16 [128,4,128] f16."""
            q4 = qe.rearrange("p (h d) -> p h d", d=D)
            sq = tmp_pool.tile([128, 4, D], F32, tag="sq4")
            nc.vector.tensor_mul(sq, q4, q4)
            ssum = tmp_pool.tile([128, 4], F32, tag="ssum4")
            nc.vector.tensor_reduce(ssum, sq, axis=mybir.AxisListType.X,
                                    op=mybir.AluOpType.add)
            nc.scalar.activation(out=ssum, in_=ssum,
                                 func=mybir.ActivationFunctionType.Sqrt,
                                 bias=epst, scale=1.0 / D)
            nc.vector.reciprocal(ssum, ssum)
            sb = tmp_pool.tile([128, 4, D], F32, tag="sb4")
            nc.vector.tensor_copy(out=sb, in_=ssum[:, :, None].broadcast_to([128, 4, D]))
            nc.vector.tensor_mul(qe, qe, sb.rearrange("p h d -> p (h d)"))
            x1 = q4[:, :, 0:64]
            x2 = q4[:, :, 64:128]
            cw1 = cst[:, None, 0:64].broadcast_to([128, 4, 64])
            sw2 = cst[:, None, 64:128].broadcast_to([128, 4, 64])
            cw2 = cst[:, None, 128:192].broadcast_to([128, 4, 64])
            sw1 = cst[:, None, 192:256].broadcast_to([128, 4, 64])
            t1 = tmp_pool.tile([128, 4, 64], F32, tag="t14")
            t2 = tmp_pool.tile([128, 4, 64], F32, tag="t24")
            nc.vector.tensor_mul(t1, x1, cw1)
            nc.vector.tensor_mul(t2, x2, sw2)
            nc.vector.tensor_sub(ro16[:, :, 0:64], t1, t2)
            nc.vector.tensor_mul(t1, x2, cw2)
            nc.vector.tensor_mul(t2, x1, sw1)
            nc.vector.tensor_add(ro16[:, :, 64:128], t1, t2)

        # ---- phase 1: query columns -> Q proj (all 4 heads) + KV rows 32+qc ----
        for qc in range(NQC):
            ckt = ck_pool.tile([128, NHD, 128], F16, tag="ck")
            nc.sync.dma_start(out=ckt, in_=ckT[NKV - NQC + qc])
            pq = psum.tile([128, HPC * D], F32, tag="acc", name=f"pq{qc}")
            pk = psum.tile([128, 2 * D], F32, tag="acc", name=f"pk{qc}")
            for k in range(NHD):
                nc.tensor.matmul(pq, lhsT=ckt[:, k, :], rhs=wq_sb[:, k, :],
                                 start=(k == 0), stop=(k == NHD - 1))
            for k in range(NHD):
                nc.tensor.matmul(pk, lhsT=ckt[:, k, :], rhs=wkv_sb[:, k, :],
                                 start=(k == 0), stop=(k == NHD - 1))
            r = NKV - NQC + qc  # kv chunk index of this query block

            cstq = cs_pool.tile([128, 4 * 64], F32, tag="csq")
            nc.scalar.dma_start(out=cstq, in_=csq[qc * 128:(qc + 1) * 128, :])
            cstk = cs_pool.tile([128, 4 * 64], F32, tag="csk")
            nc.scalar.dma_start(out=cstk, in_=csk[r * 128:(r + 1) * 128, :])

            qe = evac_pool.tile([128, HPC * D], F32, tag="qe")
            nc.scalar.copy(out=qe, in_=pq)
            ke = evac_pool.tile([128, 2 * D], F32, tag="ke")
            nc.scalar.copy(out=ke, in_=pk)

            ro4 = tmp_pool.tile([128, 4, 128], F16, tag="ro4")
            rmsnorm_rope_q4(qe, cstq, ro4)
            tpq = psum.tile([128, 512], F16, tag="tp", bufs=1, name=f"tpq{qc}")
            for h in range(HPC):
                nc.tensor.transpose(tpq[:, h * 128:(h + 1) * 128], ro4[:, h, :], ident16)
                nc.vector.tensor_copy(out=qT_sb[:, h, qc * 128:(qc + 1) * 128],
                                      in_=tpq[:, h * 128:(h + 1) * 128])
            rok = tmp_pool.tile([128, 128], F16, tag="rok")
            rmsnorm_rope_k(ke, cstk, rok)
            tpk = psum.tile([128, 512], F16, tag="tp", bufs=1, name=f"tpk{qc}")
            nc.tensor.transpose(tpk[:, 0:128], rok, ident16)
            nc.vector.tensor_copy(out=kT_sb[:, r * 128:(r + 1) * 128], in_=tpk[:, 0:128])
            nc.vector.tensor_copy(out=v_sb[:, r, :], in_=ke[:, D:2 * D])

        # ---- phase 2: context rows -> K/V chunks r=0..31 ----
        for r in range(NKV - NQC):
            ckt = ck_pool.tile([128, NHD, 128], F16, tag="ck")
            nc.sync.dma_start(out=ckt, in_=ckT[r])
            pk = psum.tile([128, 2 * D], F32, tag="acc", name=f"pkc{r}")
            for k in range(NHD):
                nc.tensor.matmul(pk, lhsT=ckt[:, k, :], rhs=wkv_sb[:, k, :],
                                 start=(k == 0), stop=(k == NHD - 1))
            cstk = cs_pool.tile([128, 4 * 64], F32, tag="csk")
            nc.scalar.dma_start(out=cstk, in_=csk[r * 128:(r + 1) * 128, :])
            ke = evac_pool.tile([128, 2 * D], F32, tag="ke")
            nc.scalar.copy(out=ke, in_=pk)
            rok = tmp_pool.tile([128, 128], F16, tag="rok")
            rmsnorm_rope_k(ke, cstk, rok)
            tpk = psum.tile([128, 512], F16, tag="tp", bufs=1, name=f"tpc{r}")
            nc.tensor.transpose(tpk[:, 0:128], rok, ident16)
            nc.vector.tensor_copy(out=kT_sb[:, r * 128:(r + 1) * 128], in_=tpk[:, 0:128])
            nc.vector.tensor_copy(out=v_sb[:, r, :], in_=ke[:, D:2 * D])

        # ---- phase 3: attention strips (j outer, h inner) + interleaved o_proj ----
        # S^T orientation: [kv partitions, q free]; exp output IS P^T; PV with
        # V stationary gives out^T [d, q].  kv chunk r fully visible iff
        # r <= 31+4j, partial (mask i=r-32-4j) for i in 0..3.
        def do_oproj(qc):
            po = psum.tile([128, HPC * D], F32, tag="acc", name=f"po{qc}")
            jq, qo = qc // 4, (qc % 4) * 128
            for h in range(HPC):
                at = oproj_pool.tile([128, NCORES, 128], F16, tag="at")
                nc.sync.dma_start(
                    out=at,
                    in_=ag_outs[h][jq][:, qo:qo + 128].rearrange(
                        "(c p) q -> p c q", p=128))
                for ci in range(NCORES):
                    nc.tensor.matmul(po, lhsT=at[:, ci, :], rhs=wo_sb[:, 4 * ci + h, :],
                                     start=(h == 0 and ci == 0),
                                     stop=(h == HPC - 1 and ci == NCORES - 1))
            ot = stg_pool.tile([128, HPC * D], F32, tag="ot")
            nc.vector.tensor_copy(out=ot, in_=po)
            nc.sync.dma_start(out=out_ext[qc * 128:(qc + 1) * 128, :], in_=ot)

        def do_strip(h, j):
            n_r = 36 + 4 * j
            o_acc = psum.tile([128, 512], F32, tag="acc", name=f"oacc{h}_{j}")
            sacc = sacc_pool.tile([128, 2, 512], F16, tag="sacc", name=f"sacc{h}_{j}")
            qs = qT_sb[:, h, j * 512:(j + 1) * 512]
            for b in range(n_r // 2):
                r0 = 2 * b
                st2 = psum.tile([128, 2, 512], F32, tag="st", bufs=2,
                                name=f"st{h}_{j}_{b}")
                for i in range(2):
                    nc.tensor.matmul(st2[:, i, :], lhsT=kT_sb[:, (r0 + i) * 128:(r0 + i + 1) * 128],
                                     rhs=qs, start=True, stop=True)
                pt2 = pt_pool.tile([128, 2, 512], F16, tag="pt")
                nc.scalar.activation(out=pt2.rearrange("p i c -> p (i c)"),
                                     in_=st2.rearrange("p i c -> p (i c)"),
                                     func=mybir.ActivationFunctionType.Exp,
                                     bias=zbias, scale=SCALE)
                i0 = r0 - 32 - 4 * j  # mask index of first tile in batch
                if i0 >= 0:
                    nc.vector.tensor_mul(pt2.rearrange("p i c -> p (i c)"),
                                         pt2.rearrange("p i c -> p (i c)"),
                                         msk_sb[:, i0:i0 + 2, :].rearrange("p i c -> p (i c)"))
                if b == 0:
                    nc.vector.tensor_copy(out=sacc, in_=pt2)
                else:
                    nc.vector.tensor_add(sacc.rearrange("p i c -> p (i c)"),
                                         sacc.rearrange("p i c -> p (i c)"),
                                         pt2.rearrange("p i c -> p (i c)"))
                for i in range(2):
                    nc.tensor.matmul(o_acc, lhsT=v_sb[:, r0 + i, :], rhs=pt2[:, i, :],
                                     start=(r0 + i == 0), stop=(r0 + i == n_r - 1))
            # denominator: broadcast column sums via ones-matmul, then normalize
            denom = psum.tile([128, 512], F32, tag="tp", bufs=1, name=f"den{h}_{j}")
            for i in range(2):
                nc.tensor.matmul(denom, lhsT=ones128, rhs=sacc[:, i, :],
                                 start=(i == 0), stop=(i == 1))
            pr = stg_pool.tile([128, 512], F32, tag="pr")
            nc.vector.reciprocal(pr, denom)
            stg = stg_pool.tile([128, 512], F16, tag="stg")
            nc.vector.tensor_mul(stg, o_acc, pr)
            nc.sync.dma_start(out=ag_ins[h][j][:], in_=stg)
            nc.gpsimd.collective_compute(
                "AllGather",
                mybir.AluOpType.bypass,
                ins=[ag_ins[h][j][:]],
                outs=[ag_outs[h][j][:]],
                replica_groups=[list(range(NCORES))],
            )

        for j in range(NQJ):
            for h in range(HPC):
                do_strip(h, j)
            # o_proj for the previous j-group while this j's AllGathers fly
            if j >= 1:
                for qc in range(4 * (j - 1), 4 * j):
                    do_oproj(qc)
        for qc in range(4 * (NQJ - 1), NQC):
            do_oproj(qc)

    nc.compile()
    return nc


def _host_prep(context, query, w_qkv, w_o, q_norm_w, k_norm_w):
    context = np.asarray(context, dtype=np.float32)
    query = np.asarray(query, dtype=np.float32)
    w_qkv = np.asarray(w_qkv, dtype=np.float32)
    w_o = np.asarray(w_o, dtype=np.float32)
    q_norm_w = np.asarray(q_norm_w, dtype=np.float32)
    k_norm_w = np.asarray(k_norm_w, dtype=np.float32)

    ck = np.concatenate([context, query], axis=0)  # [KV, HID]
    # chunk-major ckT[r, p, k, c] = ck[r*128+c, k*128+p]
    ckT = np.ascontiguousarray(
        ck.astype(np.float16).reshape(NKV, 128, NHD, 128).transpose(0, 3, 2, 1))

    wq = w_qkv[:, :H * D]
    wk = w_qkv[:, H * D:H * D + KVH * D]
    wv = w_qkv[:, H * D + KVH * D:]

    half = D // 2
    inv_freq = (1.0 / (THETA ** (np.arange(0, half, dtype=np.float32) / half))).astype(np.float32)
    pos = np.arange(KV, dtype=np.float32)
    freqs = pos[:, None] * inv_freq[None, :]
    cosf = np.cos(freqs)
    sinf = np.sin(freqs)

    def cs4(w):
        w1 = w[:half][None, :]
        w2 = w[half:][None, :]
        return np.concatenate([cosf * w1, sinf * w2, cosf * w2, sinf * w1],
                              axis=1).astype(np.float32)  # [KV, 256]

    csq = np.ascontiguousarray(cs4(q_norm_w)[CTX:])  # [QL, 256]
    csk = cs4(k_norm_w)  # [KV, 256]

    p = np.arange(128)[:, None]
    q = np.arange(512)[None, :]
    msk = np.concatenate(
        [np.where(128 * i + p <= q, 1.0, 0.0) for i in range(4)],
        axis=1).astype(np.float16)  # [128, 2048]

    def wshuf(w):
        # [HID, n] -> [128, NHD, n] with w[k*128+p, :] at [p, k, :]
        n = w.shape[1]
        return np.ascontiguousarray(
            w.astype(np.float16).reshape(NHD, 128, n).transpose(1, 0, 2))

    in_maps = []
    for c in range(NCORES):
        in_maps.append({
            "ckT": ckT,
            "wq": wshuf(wq[:, c * HPC * D:(c + 1) * HPC * D]),
            "wkv": wshuf(np.concatenate(
                [wk[:, c * D:(c + 1) * D], wv[:, c * D:(c + 1) * D]], axis=1)),
            "wo": wshuf(w_o[:, c * HPC * D:(c + 1) * HPC * D]),
            "csq": csq,
            "csk": csk,
            "msk": msk,
        })
    return in_maps


def kernel(context, query, w_qkv, w_o, q_norm_w, k_norm_w, **kw):
    if "nc" not in _STATE:
        _STATE["nc"] = _build()
    nc = _STATE["nc"]
    in_maps = _host_prep(context, query, w_qkv, w_o, q_norm_w, k_norm_w)
    res = run_bass_kernel_spmd(nc, in_maps, list(range(NCORES)), **kw)
    out = np.concatenate([np.asarray(res.results[c]["out"]) for c in range(NCORES)], axis=1)
    if kw:
        return out.astype(np.float32), res
    return out.astype(np.float32)
